# revision 38
# baseline (speedup 1.0000x reference)
"""MoE (E=64, K=8, D=512, I=1024, C=1024) on 8 TRN2 NeuronCores.

Strategy (expert-parallel, per sharding hint):
  - Host: gating (scores/softmax/top-k), dispatch bookkeeping (stable sort by
    expert, capacity slots) and packing of the per-core dispatch buffers.
    All device tensors are packed on the host into the exact SBUF tile
    layouts so every DMA is a fully contiguous 2D copy.
  - Device (SPMD, 8 cores, 8 experts/core): grouped SwiGLU expert GEMMs.
    Stage 1 (x@w1, x@w3) runs in bf16; stage 2 (h@w2) runs in fp8-e4m3
    DoubleRow mode (2x PE throughput, half the w2 HBM bytes).  Scale folding
    keeps fp8 in range: w3 is pre-scaled by 8 (so h = 8*h_true), w2 by 64;
    the host divides the combine weights by 512.  A data-parallel shard of
    the shared expert runs in bf16.
  - Host: weighted combine of expert outputs back to token order + shared
    expert add.

kernel(**inputs) takes the FULL unsharded inputs and returns the FULL
[B, S, D] float32 output.
"""

import sys

for _p in ("/opt/trn_rl_repo",):
    if _p not in sys.path:
        sys.path.append(_p)

import numpy as np
import ml_dtypes

import concourse.bacc as bacc
import concourse.mybir as mybir
import concourse.tile as tile
from concourse.bass_utils import run_bass_kernel_spmd

E = 64          # experts
K = 8           # top-k
D = 512         # model dim
I = 1024        # expert inner dim
CAP = 1024      # per-expert capacity in the reference
NCORES = 8
EL = E // NCORES  # experts per core (8)

BF16 = mybir.dt.bfloat16
F8 = mybir.dt.float8e4
F32 = mybir.dt.float32
NPF8 = ml_dtypes.float8_e4m3

SCALE_H = 8.0    # folded into w3 on host
SCALE_W2 = 64.0  # folded into w2 on host
DESCALE = 1.0 / (SCALE_H * SCALE_W2)

# set by test harness: when True, kernel() profiles the NEFF and stores
# exec_time_ns in LAST_EXEC_TIME_NS
TRACE = False
LAST_EXEC_TIME_NS = None
LAST_PROFILE = None

_KERNEL_CACHE = {}


def _install_ntff_hook():
    """antenv.axon_hooks shim so trace=True works under axon here."""
    import types

    try:
        from antenv.axon_hooks import get_axon_ntff_profile_hook  # noqa: F401
    except ImportError:
        import antenv

        m = types.ModuleType("antenv.axon_hooks")
        _store = {}
        m.set_axon_ntff_profile_hook = lambda h: _store.__setitem__("h", h)
        m.get_axon_ntff_profile_hook = lambda: _store.get("h")
        sys.modules["antenv.axon_hooks"] = m
        antenv.axon_hooks = m
    from antenv.axon_hooks import (
        get_axon_ntff_profile_hook,
        set_axon_ntff_profile_hook,
    )

    if get_axon_ntff_profile_hook() is None:
        from trn_agent_boot.trn_boot import _ntff_profile_via_ctypes

        set_axon_ntff_profile_hook(
            _ntff_profile_via_ctypes("/opt/axon/libaxon_pjrt.so")
        )
    from concourse import bass_utils

    bass_utils.upload_artifacts = lambda tmpdir: f"local://{tmpdir}"


def _stage1(nc, pools, wsb, x_t, xcol0, n_tok, fp8_h):
    """Emit h = silu(x@w1) * (x@w3) for n_tok tokens at column xcol0.

    wsb: [128, 8, 2, 512] weight tile ([p, j, b, t*128+c])
    x_t: [128, 4, NTOK] token tile (d-major)
    Returns the list of h tiles: 4 pair tiles [128, 2, n_tok] fp8 when
    fp8_h, else 8 tiles [128, n_tok] bf16.
    """
    psum_pool, _, h_pool, s_pool, _ = pools
    n_d = D // 128   # 4
    n_i = I // 128   # 8

    h_tiles = []
    for j in range(n_i):
        ps1 = psum_pool.tile([128, n_tok], F32, tag="ps1")
        ps3 = psum_pool.tile([128, n_tok], F32, tag="ps3")
        for b, ps in ((0, ps1), (1, ps3)):
            for t in range(n_d):
                nc.tensor.matmul(
                    ps[:],
                    wsb[:, j, b, t * 128 : (t + 1) * 128],
                    x_t[:, t, xcol0 : xcol0 + n_tok],
                    start=(t == 0),
                    stop=(t == n_d - 1),
                )
        sil = s_pool.tile([128, n_tok], F32, tag="sil")
        nc.scalar.activation(sil[:], ps1[:], mybir.ActivationFunctionType.Silu)
        if fp8_h:
            if j % 2 == 0:
                hp = h_pool.tile([128, 2, n_tok], F8, tag=f"hp{j // 2}")
                h_tiles.append(hp)
            nc.vector.tensor_mul(h_tiles[j // 2][:, j % 2, :], sil[:], ps3[:])
        else:
            h_j = h_pool.tile([128, n_tok], BF16, tag=f"hs{j}")
            nc.vector.tensor_mul(h_j[:], sil[:], ps3[:])
            h_tiles.append(h_j)
    return h_tiles


def _stage2_fp8(nc, pools, w2sb, h_tiles, n_tok, store):
    """y = h @ w2 in fp8 DoubleRow. w2sb: [128, 8, 512] fp8.
    Collects the 4 output d-tiles into one [128, 4, n_tok] tile, then
    hands it to `store` for a single consolidated DMA."""
    _, psumy_pool, _, _, y_pool = pools
    yb = y_pool.tile([128, 4, n_tok], BF16, tag="ysb")
    for m2 in range(D // 128):
        psy = psumy_pool.tile([128, n_tok], F32, tag="psy")
        for k in range(4):
            nc.tensor.matmul(
                psy[:],
                w2sb[:, 2 * k : 2 * k + 2, m2 * 128 : (m2 + 1) * 128],
                h_tiles[k][:, :, :n_tok],
                start=(k == 0),
                stop=(k == 3),
                perf_mode=mybir.MatmulPerfMode.DoubleRow,
            )
        nc.vector.tensor_copy(yb[:, m2, :], psy[:])
        if m2 == 1:
            store(yb, 0)
    store(yb, 1)


def _stage2_bf16(nc, pools, w2sb, h_tiles, n_tok, store):
    """y = h @ w2 in bf16 (shared expert). w2sb: [128, 8, 512] bf16."""
    _, psumy_pool, _, _, y_pool = pools
    yb = y_pool.tile([128, 4, n_tok], BF16, tag="ysb")
    for m2 in range(D // 128):
        psy = psumy_pool.tile([128, n_tok], F32, tag="psy")
        for t2 in range(I // 128):
            nc.tensor.matmul(
                psy[:],
                w2sb[:, t2, m2 * 128 : (m2 + 1) * 128],
                h_tiles[t2][:],
                start=(t2 == 0),
                stop=(t2 == 7),
            )
        nc.vector.tensor_copy(yb[:, m2, :], psy[:])
    store(yb)


def _build(caps, TS):
    """Build the SPMD Bass kernel.

    DRAM params (per core, all pre-packed in SBUF layout on host):
      xbuf [128, 4, NTOK] bf16   dispatched tokens ([p, t, col])
      w13  [EL, 128, 8, 2, 512] bf16  ([e, p, j, b, t*128+c]; w3 branch x8)
      w2p  [EL, 128, 8, 512] f8e4     ([e, p, t2, d]; x64)
      xs   [128, 4, TS] bf16 ; ws13 [128, 8, 2, 512] bf16 ; ws2 [128, 8, 512] bf16
    Outputs:
      yexp [128, 4, NTOK] bf16 (= 512*y) ; ysh [128, 4, TS] bf16
    """
    NTOK = int(sum(caps))
    offs = [0]
    for c in caps:
        offs.append(offs[-1] + int(c))
    nc = bacc.Bacc("TRN2", target_bir_lowering=False)

    xbuf = nc.declare_dram_parameter("xbuf", [128, 4 * NTOK], BF16, isOutput=False)
    w13 = nc.declare_dram_parameter("w13", [EL, 128, 8, 2, 512], BF16, isOutput=False)
    w2p = nc.declare_dram_parameter("w2p", [EL, 128, 8, 512], F8, isOutput=False)
    xs = nc.declare_dram_parameter("xs", [128, 4, TS], BF16, isOutput=False)
    ws13 = nc.declare_dram_parameter("ws13", [128, 8, 2, 512], BF16, isOutput=False)
    ws2 = nc.declare_dram_parameter("ws2", [128, 8, 512], BF16, isOutput=False)
    yexp = nc.declare_dram_parameter("yexp", [128, 4 * NTOK], BF16, isOutput=True)
    ysh = nc.declare_dram_parameter("ysh", [128, 4 * TS], BF16, isOutput=True)

    with tile.TileContext(nc) as tc:
        with (
            tc.tile_pool(name="xpool", bufs=1) as xpool,
            tc.tile_pool(name="wpool", bufs=5) as wpool,
            tc.tile_pool(name="w2pool", bufs=5) as w2pool,
            tc.tile_pool(name="wspool", bufs=1) as wspool,
            tc.tile_pool(name="hpool", bufs=2) as h_pool,
            tc.tile_pool(name="hspool", bufs=1) as hs_pool,
            tc.tile_pool(name="spool", bufs=4) as s_pool,
            tc.tile_pool(name="ypool", bufs=3) as y_pool,
            tc.tile_pool(name="psum", bufs=3, space="PSUM") as psum_pool,
            tc.tile_pool(name="psumy", bufs=2, space="PSUM") as psumy_pool,
        ):
            # ---- DMA issue.  Every transfer is a contiguous 2D copy.
            # sync (HWDGE ring A): pure expert w13 stream (expert 0 in
            # quarters for a fast PE start).  scalar (HWDGE ring B): x0/xs,
            # shared weights in chunks, per-expert w2p, shared store.
            # gpsimd (SWDGE, own semaphore lanes): remaining token slots +
            # one consolidated yexp store per expert.
            x_tiles = []
            for s in range(EL):
                xst = xpool.tile([128, 4, int(caps[s])], BF16, tag=f"x{s}")
                x_tiles.append(xst)
            # x0 in halves so the first matmul's dep is ~0.13 MB
            nc.scalar.dma_start(
                x_tiles[0][:, 0:2], xbuf[:, 4 * offs[0] : 4 * offs[0] + 2 * caps[0]]
            )
            nc.scalar.dma_start(
                x_tiles[0][:, 2:4],
                xbuf[:, 4 * offs[0] + 2 * caps[0] : 4 * offs[1]],
            )
            xs_t = xpool.tile([128, 4, TS], BF16, tag="xs")

            def issue_x(s):
                nc.gpsimd.dma_start(
                    x_tiles[s][:], xbuf[:, 4 * offs[s] : 4 * offs[s + 1]]
                )

            issue_x(1)
            issue_x(2)
            # Warm-up activation so the Silu table loads before the first
            # real silu (after the critical head DMA issues).
            warm = s_pool.tile([128, 1], F32, tag="warm")
            nc.gpsimd.memset(warm[:], 0.0)
            nc.scalar.activation(
                warm[:], warm[:], mybir.ActivationFunctionType.Silu
            )

            ws13sb = wspool.tile([128, 8, 2, 512], BF16, tag="ws13")
            ws2sb = wspool.tile([128, 8, 512], BF16, tag="ws2")

            pools = (psum_pool, psumy_pool, h_pool, s_pool, y_pool)
            sh_pools = (psum_pool, psumy_pool, hs_pool, s_pool, y_pool)

            def chunks(total):
                out = []
                s = 0
                while s < total:
                    out.append((s, min(512, total - s)))
                    s += 512
                return out

            # Block sequence: e0, shared, e1..e7.  For each block emit
            # stage1 then the previous block's stage2 (keeps the PE busy
            # while silu/mul of the current block completes).  Weight DMAs
            # for expert k+1 are emitted before block k-1's stores so the
            # sync queue never delays the weight stream behind store waits.
            pending = []
            wt = {}
            wt2 = {}

            def issue_w(le):
                wsb = wpool.tile([128, 8, 2, 512], BF16, tag="w13")
                if le == 0:
                    for q in range(8):
                        nc.sync.dma_start(wsb[:, q : q + 1], w13[le][:, q : q + 1])
                elif le <= 2:
                    for q in range(4):
                        nc.sync.dma_start(
                            wsb[:, 2 * q : 2 * q + 2], w13[le][:, 2 * q : 2 * q + 2]
                        )
                else:
                    nc.sync.dma_start(wsb[:], w13[le])
                wt[le] = wsb

            def issue_w2(le):
                w2sb = w2pool.tile([128, 8, 512], F8, tag="w2")
                nc.scalar.dma_start(w2sb[:], w2p[le])
                wt2[le] = w2sb

            def emit_expert_s1(le):
                wsb = wt[le]
                w2sb = wt2[le]
                last = le == EL - 1

                def store(yb, phase, c, n, last=last):
                    if last:
                        # split: first half streams out while the second
                        # half's psy copies finish
                        if phase == 0:
                            nc.sync.dma_start(
                                yexp[:, 4 * c : 4 * c + 2 * n], yb[:, 0:2]
                            )
                        else:
                            nc.sync.dma_start(
                                yexp[:, 4 * c + 2 * n : 4 * (c + n)], yb[:, 2:4]
                            )
                    elif phase == 1:
                        nc.gpsimd.dma_start(yexp[:, 4 * c : 4 * (c + n)], yb[:])

                for c0, cn in chunks(int(caps[le])):
                    col0 = offs[le] + c0
                    h_tiles = _stage1(nc, pools, wsb, x_tiles[le], c0, cn, fp8_h=True)
                    pending.append(
                        lambda h=h_tiles, n=cn, c=col0, w=w2sb: _stage2_fp8(
                            nc, pools, w, h, n,
                            lambda yb, phase, c=c, n=n: store(yb, phase, c, n),
                        )
                    )

            def emit_shared_s1():
                for s0, sn in chunks(TS):
                    h_tiles = _stage1(nc, sh_pools, ws13sb, xs_t, s0, sn, fp8_h=False)
                    pending.append(
                        lambda h=h_tiles, n=sn, s=s0: _stage2_bf16(
                            nc, sh_pools, ws2sb, h, n,
                            lambda yb, s=s, n=n: nc.scalar.dma_start(
                                ysh[:, 4 * s : 4 * (s + n)], yb[:]
                            ),
                        )
                    )

            # Block order: e0, e1, e2, shared, e3..e7 — the shared expert's
            # 3 MB of weights load on the scalar ring while e0-e2 run, so
            # the early HBM demand stays under the aggregate bandwidth.
            issue_w(0)
            issue_w(1)
            issue_w2(0)
            emit_expert_s1(0)
            for jc in range(2):
                nc.scalar.dma_start(
                    ws13sb[:, 2 * jc : 2 * jc + 2], ws13[:, 2 * jc : 2 * jc + 2]
                )
            issue_w(2)
            while len(pending) > 1:
                pending.pop(0)()
            issue_w2(1)
            emit_expert_s1(1)
            nc.scalar.dma_start(xs_t[:], xs[:])
            for jc in range(2, 4):
                nc.scalar.dma_start(
                    ws13sb[:, 2 * jc : 2 * jc + 2], ws13[:, 2 * jc : 2 * jc + 2]
                )
            nc.scalar.dma_start(ws2sb[:], ws2[:])
            issue_w(3)
            while len(pending) > 1:
                pending.pop(0)()
            issue_w2(2)
            issue_x(3)
            emit_expert_s1(2)
            issue_w(4)
            while len(pending) > 1:
                pending.pop(0)()
            issue_x(4)
            emit_shared_s1()
            for le in range(3, EL):
                if le + 2 < EL:
                    issue_w(le + 2)
                if le + 2 < EL:
                    issue_x(le + 2)
                while len(pending) > 1:
                    pending.pop(0)()
                issue_w2(le)
                emit_expert_s1(le)
            while pending:
                pending.pop(0)()

    nc.compile()
    return nc


def _softmax(x):
    m = x.max(axis=-1, keepdims=True)
    e = np.exp(x - m)
    return e / e.sum(axis=-1, keepdims=True)


def kernel(x, gate_w, adaptive_bias, w1, w3, w2, ws1, ws3, ws2):
    global LAST_EXEC_TIME_NS, LAST_PROFILE

    x = np.asarray(x, dtype=np.float32)
    gate_w = np.asarray(gate_w, dtype=np.float32)
    adaptive_bias = np.asarray(adaptive_bias, dtype=np.float32)
    w1 = np.asarray(w1, dtype=np.float32)
    w3 = np.asarray(w3, dtype=np.float32)
    w2 = np.asarray(w2, dtype=np.float32)
    ws1 = np.asarray(ws1, dtype=np.float32)
    ws3 = np.asarray(ws3, dtype=np.float32)
    ws2 = np.asarray(ws2, dtype=np.float32)

    B, S, _ = x.shape
    T = B * S
    xf = x.reshape(T, D)

    # ---- gating (host, fp32, mirrors reference semantics) ----
    scores = xf @ gate_w.T + adaptive_bias
    probs = _softmax(scores)
    # jax.lax.top_k == stable descending sort, lower index wins ties
    topi = np.argsort(-probs, axis=-1, kind="stable")[:, :K].astype(np.int32)
    topw = np.take_along_axis(probs, topi, axis=-1)
    topw = topw / (topw.sum(axis=-1, keepdims=True) + 1e-8)

    flat_e = topi.reshape(-1)
    flat_w = topw.reshape(-1).astype(np.float32)
    flat_t = np.repeat(np.arange(T), K)

    order = np.argsort(flat_e, kind="stable")
    counts = np.bincount(flat_e, minlength=E)
    offsets = np.cumsum(counts) - counts
    slot_sorted = np.arange(T * K) - offsets[flat_e[order]]
    slot = np.empty(T * K, np.int64)
    slot[order] = slot_sorted
    valid = slot < CAP
    eff_counts = np.minimum(counts, CAP)

    # Assign experts to (core, slot) by load rank: slot s holds the experts
    # ranked [s*NCORES, (s+1)*NCORES), one per core, so every core has the
    # same per-slot capacity with minimal padding.
    perm = np.argsort(-eff_counts, kind="stable")        # expert ids by load desc
    rank = np.empty(E, np.int64)
    rank[perm] = np.arange(E)
    core_of = rank % NCORES
    slot_of = rank // NCORES
    caps = tuple(
        max(4, (int(eff_counts[perm[s * NCORES]]) + 3) // 4 * 4)
        for s in range(EL)
    )
    offs = np.concatenate([[0], np.cumsum(caps)])
    assert T % NCORES == 0
    TS = T // NCORES

    key = (caps, TS)
    if key not in _KERNEL_CACHE:
        _KERNEL_CACHE[key] = _build(caps, TS)
    nc = _KERNEL_CACHE[key]

    # ---- pack per-core inputs (exact SBUF layouts) ----
    xb16 = xf.astype(ml_dtypes.bfloat16)
    # w13 packed [E, 128, 8, 2, 512]: [e, p, j, b, t*128+c]; w3 branch x8
    w1r = w1.astype(ml_dtypes.bfloat16).reshape(E, 4, 128, 8, 128)
    w3r = (w3 * SCALE_H).astype(ml_dtypes.bfloat16).reshape(E, 4, 128, 8, 128)
    w13_all = np.stack(
        [w1r.transpose(0, 2, 3, 1, 4), w3r.transpose(0, 2, 3, 1, 4)], axis=3
    ).reshape(E, 128, 8, 2, 512)
    # w2 packed [E, 128, 8, 512] fp8: [e, p, t2, d]; x64
    w2p_all = np.ascontiguousarray(
        (w2 * SCALE_W2).astype(NPF8).reshape(E, 8, 128, 512).transpose(0, 2, 1, 3)
    )
    # shared weights (unscaled, bf16)
    ws1r = ws1.astype(ml_dtypes.bfloat16).reshape(4, 128, 8, 128)
    ws3r = ws3.astype(ml_dtypes.bfloat16).reshape(4, 128, 8, 128)
    ws13_p = np.ascontiguousarray(
        np.stack(
            [ws1r.transpose(1, 2, 0, 3), ws3r.transpose(1, 2, 0, 3)], axis=2
        ).reshape(128, 8, 2, 512)
    )
    ws2_p = np.ascontiguousarray(
        ws2.astype(ml_dtypes.bfloat16).reshape(8, 128, 512).transpose(1, 0, 2)
    )

    NTOK = int(sum(caps))
    v_idx = np.where(valid)[0]
    v_e = flat_e[v_idx]
    v_t = flat_t[v_idx]
    v_slot = slot[v_idx]
    v_core = core_of[v_e]
    v_col = offs[slot_of[v_e]] + v_slot  # column in that core's dispatch buffer

    in_maps = []
    for c in range(NCORES):
        m = v_core == c
        xbuf_c = np.zeros((NTOK, D), dtype=ml_dtypes.bfloat16)
        xbuf_c[v_col[m]] = xb16[v_t[m]]
        # per-slot contiguous blocks: [128, 4*cap_s] at columns 4*offs[s]
        xbuf_p = np.empty((128, 4 * NTOK), dtype=ml_dtypes.bfloat16)
        for s in range(EL):
            blk = xbuf_c[offs[s] : offs[s + 1]].reshape(caps[s], 4, 128)
            xbuf_p[:, 4 * offs[s] : 4 * offs[s + 1]] = blk.transpose(2, 1, 0).reshape(
                128, 4 * caps[s]
            )
        experts_c = perm[np.arange(EL) * NCORES + c]  # slot s -> expert id
        in_maps.append(
            {
                "xbuf": xbuf_p,
                "w13": np.ascontiguousarray(w13_all[experts_c]),
                "w2p": w2p_all[experts_c],
                "xs": np.ascontiguousarray(
                    xb16[c * TS : (c + 1) * TS].reshape(TS, 4, 128).transpose(2, 1, 0)
                ),
                "ws13": ws13_p,
                "ws2": ws2_p,
            }
        )

    # ---- run on 8 cores ----
    if TRACE:
        _install_ntff_hook()
    res = run_bass_kernel_spmd(
        nc, in_maps, core_ids=list(range(NCORES)), trace=TRACE
    )
    LAST_EXEC_TIME_NS = res.exec_time_ns
    LAST_PROFILE = res
    # yexp per core: [128, 4*NTOK] bf16 (= 512*y, per-slot blocks) ; ysh: [128, 4*TS]
    yexp_l = []
    for c in range(NCORES):
        a = res.results[c]["yexp"].astype(np.float32)  # [128, 4*NTOK]
        out_c = np.empty((NTOK, D), np.float32)
        for s in range(EL):
            blk = a[:, 4 * offs[s] : 4 * offs[s + 1]].reshape(128, 4, caps[s])
            out_c[offs[s] : offs[s + 1]] = blk.transpose(2, 1, 0).reshape(caps[s], D)
        yexp_l.append(out_c)
    yexp = np.stack(yexp_l)
    ysh = np.stack(
        [
            res.results[c]["ysh"]
            .astype(np.float32)
            .reshape(128, 4, TS)
            .transpose(2, 1, 0)
            .reshape(TS, D)
            for c in range(NCORES)
        ]
    )

    # ---- combine on host ----
    pair_y = np.zeros((T * K, D), np.float32)
    pair_y[v_idx] = yexp[v_core, v_col]  # gather [n_valid, D]
    w_eff = flat_w * valid.astype(np.float32) * DESCALE
    out = (pair_y * w_eff[:, None]).reshape(T, K, D).sum(axis=1)

    shared = ysh.reshape(T, D)
    out = out + shared
    return out.reshape(B, S, D).astype(np.float32)
